# revision 4
# baseline (speedup 1.0000x reference)
"""Trainium2 Bass kernel for nn_CorresAttention_66554813219085.

Mathematical analysis of the module (exact arithmetic):

1. ``x_f = sum_k(softmax_k(feat))`` sums a softmax over the axis it
   normalizes, so ``x_f == 1`` identically — the entire KNN search,
   gather and neighbor softmax contribute nothing to the output.
2. With ``x_f`` constant, the attention keys/values are constant across
   sequence positions, so every attention row is a constant vector,
   its softmax is exactly uniform, and ``u_f = attn @ v`` collapses to
   the same constant vector at every (b, n).
3. conv1 then produces one constant scalar per position, so the
   LayerNorm over (1, N) sees zero variance and outputs exactly
   ``ln_b`` at every position.
4. The remaining pointwise tail is the only thing that survives:

       out[b, n] = sigmoid(gelu(ln_b[0, n]) * conv2_w[0, 0] + conv2_b[0])

   broadcast over the batch.  (For the shipped parameters ln_b = 0,
   conv2_b = 0, so out == 0.5 everywhere; the reference's deviations
   from this, ~5e-6, are float32 rounding noise amplified by the
   1/sqrt(var + 1e-5) normalization with var ~ 0 — backend-specific
   accumulation noise that no re-implementation can track.)

The kernel computes the exact-math tail on device.  Sharding follows
the data-parallel hint: core i produces the output rows for batches
[4*i, 4*i+4); the tiny params are replicated to all 8 cores and there
is no cross-device communication.
"""

import numpy as np

B, N = 32, 512
N_CORES = 8
B_PER_CORE = B // N_CORES

_nc_cache = []


def _build_bass():
    import concourse.bacc as bacc
    import concourse.mybir as mybir
    from concourse.tile import TileContext

    f32 = mybir.dt.float32
    nc = bacc.Bacc("TRN2", target_bir_lowering=False, debug=False)
    ln_b = nc.dram_tensor("ln_b", (1, N), f32, kind="ExternalInput")
    c2w = nc.dram_tensor("conv2_w", (1, 1), f32, kind="ExternalInput")
    c2b = nc.dram_tensor("conv2_b", (1,), f32, kind="ExternalInput")
    out = nc.dram_tensor("out", (B_PER_CORE, N), f32, kind="ExternalOutput")

    with TileContext(nc) as tc:
        with tc.tile_pool(name="p", bufs=1) as pool:
            row = pool.tile([1, N], f32)
            w1 = pool.tile([1, 1], f32)
            b1 = pool.tile([1, 1], f32)
            nc.sync.dma_start(row[:, :], ln_b[:, :])
            nc.sync.dma_start(w1[:, :], c2w[:, :])
            nc.sync.dma_start(b1[:, :], c2b[:])

            g = pool.tile([1, N], f32)
            nc.scalar.activation(g[:, :], row[:, :], mybir.ActivationFunctionType.Gelu)
            t1 = pool.tile([1, N], f32)
            nc.vector.tensor_scalar_mul(t1[:, :], g[:, :], w1[:1, :1])
            t2 = pool.tile([1, N], f32)
            nc.vector.tensor_scalar_add(t2[:, :], t1[:, :], b1[:1, :1])
            o = pool.tile([1, N], f32)
            nc.scalar.activation(
                o[:, :], t2[:, :], mybir.ActivationFunctionType.Sigmoid
            )
            for b in range(B_PER_CORE):
                nc.sync.dma_start(out[b : b + 1, :], o[:, :])
    nc.compile()
    return nc


def _get_nc():
    if not _nc_cache:
        _nc_cache.append(_build_bass())
    return _nc_cache[0]


def run_spmd(inputs, **spmd_kwargs):
    """Run the sharded kernel on all 8 cores; returns (full_out, results obj)."""
    from concourse.bass_utils import run_bass_kernel_spmd

    nc = _get_nc()
    ln_b = np.ascontiguousarray(np.asarray(inputs["ln_b"], np.float32).reshape(1, N))
    c2w = np.ascontiguousarray(np.asarray(inputs["conv2_w"], np.float32).reshape(1, 1))
    c2b = np.ascontiguousarray(np.asarray(inputs["conv2_b"], np.float32).reshape(1))
    in_map = {"ln_b": ln_b, "conv2_w": c2w, "conv2_b": c2b}
    res = run_bass_kernel_spmd(
        nc,
        [dict(in_map) for _ in range(N_CORES)],
        core_ids=list(range(N_CORES)),
        **spmd_kwargs,
    )
    full = np.concatenate([r["out"] for r in res.results], axis=0)
    return full.astype(np.float32, copy=False), res


def kernel(**inputs) -> np.ndarray:
    out, _ = run_spmd(inputs)
    return out


# revision 6
# speedup vs baseline: 1.0607x; 1.0607x over previous
"""Trainium2 Bass kernel for nn_CorresAttention_66554813219085.

Mathematical analysis of the module (exact arithmetic):

1. ``x_f = sum_k(softmax_k(feat))`` sums a softmax over the axis it
   normalizes, so ``x_f == 1`` identically — the entire KNN search,
   gather and neighbor softmax contribute nothing to the output.
2. With ``x_f`` constant, the attention keys/values are constant across
   sequence positions, so every attention row is a constant vector,
   its softmax is exactly uniform, and ``u_f = attn @ v`` collapses to
   the same constant vector at every (b, n).
3. conv1 then produces one constant scalar per position, so the
   LayerNorm over (1, N) sees zero variance and outputs exactly
   ``ln_b`` at every position.
4. The remaining pointwise tail is the only thing that survives:

       out[b, n] = sigmoid(gelu(ln_b[0, n]) * conv2_w[0, 0] + conv2_b[0])

   broadcast over the batch.  (For the shipped parameters ln_b = 0,
   conv2_b = 0, so out == 0.5 everywhere; the reference's deviations
   from this, ~5e-6, are float32 rounding noise amplified by the
   1/sqrt(var + 1e-5) normalization with var ~ 0 — backend-specific
   accumulation noise that no re-implementation can track.)

The kernel computes the exact-math tail on device.  Exact gelu is
evaluated via Erf — gelu(z) = 0.5*z*(1 + erf(z/sqrt(2))) — because Erf
and Sigmoid live in the same ACT function table ("sigmoid_and_others"),
so the scalar engine loads one table instead of two.

Layout: the 512 positions are spread over 128 partitions x 4 elements.
The host packs one (128, 7) input block per core: cols 0-3 = ln_b,
col 4 = conv2_w/2, col 5 = conv2_b, col 6 = 0 (bias operand), so a
single DMA brings in everything.  One broadcast DMA (step-0 AP) writes
the identical 4 batch rows of the (4, 512) per-core shard.

Sharding follows the data-parallel hint: core i produces the output
rows for batches [4*i, 4*i+4); the tiny params are replicated to all
8 cores and there is no cross-device communication.
"""

import numpy as np

B, N = 32, 512
N_CORES = 8
B_PER_CORE = B // N_CORES
P = 128
F = N // P  # 4 elements per partition

_nc_cache = []


def _build_bass():
    import concourse.bacc as bacc
    import concourse.mybir as mybir
    from concourse.tile import TileContext

    f32 = mybir.dt.float32
    nc = bacc.Bacc("TRN2", target_bir_lowering=False, debug=False)
    params = nc.dram_tensor("params", (P, F + 3), f32, kind="ExternalInput")
    out = nc.dram_tensor("out", (B_PER_CORE, N), f32, kind="ExternalOutput")

    with TileContext(nc) as tc:
        with tc.tile_pool(name="p", bufs=1) as pool:
            pt = pool.tile([P, F + 3], f32)
            nc.sync.dma_start(pt[:, :], params[:, :])
            zt = pt[:, 0:F]
            w_ap = pt[:, F : F + 1]  # conv2_w / 2
            b_ap = pt[:, F + 1 : F + 2]  # conv2_b
            zero_ap = pt[:, F + 2 : F + 3]  # 0.0 bias operand

            et = pool.tile([P, F], f32)
            # erf(z / sqrt(2))
            nc.scalar.activation(
                et[:, :],
                zt,
                mybir.ActivationFunctionType.Erf,
                bias=zero_ap,
                scale=0.7071067811865476,
            )
            st = pool.tile([P, F], f32)
            nc.vector.tensor_tensor(st[:, :], zt, et[:, :], mybir.AluOpType.mult)
            ut = pool.tile([P, F], f32)
            nc.vector.tensor_tensor(ut[:, :], zt, st[:, :], mybir.AluOpType.add)
            ot = pool.tile([P, F], f32)
            # sigmoid((z + z*erf) * conv2_w/2 + conv2_b) = sigmoid(gelu(z)*conv2_w + conv2_b)
            nc.scalar.activation(
                ot[:, :],
                ut[:, :],
                mybir.ActivationFunctionType.Sigmoid,
                bias=b_ap,
                scale=w_ap,
            )
            # one DMA writes all 4 identical batch rows: SBUF (p, f) read 4x
            # via a step-0 AP against DRAM ordered (p, f, b)
            nc.sync.dma_start(
                out[:, :].rearrange("b (p f) -> p b f", p=P),
                ot[:, :].to_broadcast((P, F, B_PER_CORE)).transpose([0, 2, 1]),
            )
    nc.compile()
    return nc


def _get_nc():
    if not _nc_cache:
        _nc_cache.append(_build_bass())
    return _nc_cache[0]


def _pack_params(inputs):
    ln_b = np.asarray(inputs["ln_b"], np.float32).reshape(N)
    c2w = np.asarray(inputs["conv2_w"], np.float32).reshape(())
    c2b = np.asarray(inputs["conv2_b"], np.float32).reshape(())
    pk = np.empty((P, F + 3), np.float32)
    pk[:, 0:F] = ln_b.reshape(P, F)
    pk[:, F] = c2w * 0.5
    pk[:, F + 1] = c2b
    pk[:, F + 2] = 0.0
    return pk


def run_spmd(inputs, **spmd_kwargs):
    """Run the sharded kernel on all 8 cores; returns (full_out, results obj)."""
    from concourse.bass_utils import run_bass_kernel_spmd

    nc = _get_nc()
    in_map = {"params": _pack_params(inputs)}
    res = run_bass_kernel_spmd(
        nc,
        [dict(in_map) for _ in range(N_CORES)],
        core_ids=list(range(N_CORES)),
        **spmd_kwargs,
    )
    full = np.concatenate([r["out"] for r in res.results], axis=0)
    return full.astype(np.float32, copy=False), res


def kernel(**inputs) -> np.ndarray:
    out, _ = run_spmd(inputs)
    return out


# revision 7
# speedup vs baseline: 1.0870x; 1.0248x over previous
"""Trainium2 Bass kernel for nn_CorresAttention_66554813219085.

Mathematical analysis of the module (exact arithmetic):

1. ``x_f = sum_k(softmax_k(feat))`` sums a softmax over the axis it
   normalizes, so ``x_f == 1`` identically — the entire KNN search,
   gather and neighbor softmax contribute nothing to the output.
2. With ``x_f`` constant, the attention keys/values are constant across
   sequence positions, so every attention row is a constant vector,
   its softmax is exactly uniform, and ``u_f = attn @ v`` collapses to
   the same constant vector at every (b, n).
3. conv1 then produces one constant scalar per position, so the
   LayerNorm over (1, N) sees zero variance and outputs exactly
   ``ln_b`` at every position.
4. The remaining pointwise tail is the only thing that survives:

       out[b, n] = sigmoid(gelu(ln_b[0, n]) * conv2_w[0, 0] + conv2_b[0])

   broadcast over the batch.  (For the shipped parameters ln_b = 0,
   conv2_b = 0, so out == 0.5 everywhere; the reference's deviations
   from this, ~5e-6, are float32 rounding noise amplified by the
   1/sqrt(var + 1e-5) normalization with var ~ 0 — backend-specific
   accumulation noise that no re-implementation can track.)

The kernel computes the exact-math tail on device.  Exact gelu is
evaluated via Erf — gelu(z) = 0.5*z*(1 + erf(z/sqrt(2))) — because Erf
and Sigmoid live in the same ACT function table ("sigmoid_and_others"),
so the scalar engine loads one table instead of two.

Layout: the 512 positions are spread over 128 partitions x 4 elements.
The host packs one (128, 7) input block per core: cols 0-3 = ln_b,
col 4 = conv2_w/2, col 5 = conv2_b, col 6 = 0 (bias operand), so a
single DMA brings in everything.  One broadcast DMA (step-0 AP) writes
the identical 4 batch rows of the (4, 512) per-core shard.

Sharding follows the data-parallel hint: core i produces the output
rows for batches [4*i, 4*i+4); the tiny params are replicated to all
8 cores and there is no cross-device communication.
"""

import numpy as np

B, N = 32, 512
N_CORES = 8
B_PER_CORE = B // N_CORES
P = 128
F = N // P  # 4 elements per partition

_nc_cache = []


def _build_bass():
    import concourse.bacc as bacc
    import concourse.mybir as mybir
    from concourse.tile import TileContext

    f32 = mybir.dt.float32
    nc = bacc.Bacc("TRN2", target_bir_lowering=False, debug=False)
    params = nc.dram_tensor("params", (P, F + 3), f32, kind="ExternalInput")
    out = nc.dram_tensor("out", (B_PER_CORE, N), f32, kind="ExternalOutput")

    with TileContext(nc) as tc:
        with tc.tile_pool(name="p", bufs=1) as pool:
            pt = pool.tile([P, F + 3], f32)
            nc.sync.dma_start(pt[:, :], params[:, :])
            zt = pt[:, 0:F]
            w_ap = pt[:, F : F + 1]  # conv2_w / 2
            b_ap = pt[:, F + 1 : F + 2]  # conv2_b
            zero_ap = pt[:, F + 2 : F + 3]  # 0.0 bias operand

            et = pool.tile([P, F], f32)
            # erf(z / sqrt(2))
            nc.scalar.activation(
                et[:, :],
                zt,
                mybir.ActivationFunctionType.Erf,
                bias=zero_ap,
                scale=0.7071067811865476,
            )
            st = pool.tile([P, F], f32)
            nc.vector.tensor_tensor(st[:, :], zt, et[:, :], mybir.AluOpType.mult)
            ut = pool.tile([P, F], f32)
            nc.vector.tensor_tensor(ut[:, :], zt, st[:, :], mybir.AluOpType.add)
            ot = pool.tile([P, F], f32)
            # sigmoid((z + z*erf) * conv2_w/2 + conv2_b) = sigmoid(gelu(z)*conv2_w + conv2_b)
            nc.scalar.activation(
                ot[:, :],
                ut[:, :],
                mybir.ActivationFunctionType.Sigmoid,
                bias=b_ap,
                scale=w_ap,
            )
            # one DMA writes all 4 identical batch rows: SBUF (p, f) read 4x
            # via a step-0 AP against DRAM ordered (p, f, b)
            nc.sync.dma_start(
                out[:, :].rearrange("b (p f) -> p b f", p=P),
                ot[:, :].to_broadcast((P, F, B_PER_CORE)).transpose([0, 2, 1]),
            )
    _strip_unused_const_memsets(nc)
    nc.compile()
    return nc


def _strip_unused_const_memsets(nc):
    """Bass.__init__ unconditionally seeds four const-<dtype>-<val> SBUF
    tensors with GpSimd memsets at kernel start. This kernel reads none of
    them (all ACT bias/scale operands are real APs), so drop the memsets:
    they are dead work and their presence starts the profiled window
    ~1.3us before the first real instruction can issue."""
    import concourse.mybir as mybir

    read_names = set()
    memsets = []
    for func in nc.m.functions:
        for block in func.blocks:
            for inst in block.instructions:
                is_const_memset = isinstance(inst, mybir.InstMemset) and any(
                    getattr(o, "name", "").startswith("const-") for o in inst.outs
                )
                if is_const_memset:
                    memsets.append((block, inst))
                else:
                    for o in list(inst.ins) + list(inst.outs):
                        n = getattr(o, "name", "")
                        if n.startswith("const-"):
                            read_names.add(n)
    for block, inst in memsets:
        if not any(getattr(o, "name", "") in read_names for o in inst.outs):
            block.instructions.remove(inst)
            nc.inst_map.pop(inst.name, None)


def _get_nc():
    if not _nc_cache:
        _nc_cache.append(_build_bass())
    return _nc_cache[0]


def _pack_params(inputs):
    ln_b = np.asarray(inputs["ln_b"], np.float32).reshape(N)
    c2w = np.asarray(inputs["conv2_w"], np.float32).reshape(())
    c2b = np.asarray(inputs["conv2_b"], np.float32).reshape(())
    pk = np.empty((P, F + 3), np.float32)
    pk[:, 0:F] = ln_b.reshape(P, F)
    pk[:, F] = c2w * 0.5
    pk[:, F + 1] = c2b
    pk[:, F + 2] = 0.0
    return pk


def run_spmd(inputs, **spmd_kwargs):
    """Run the sharded kernel on all 8 cores; returns (full_out, results obj)."""
    from concourse.bass_utils import run_bass_kernel_spmd

    nc = _get_nc()
    in_map = {"params": _pack_params(inputs)}
    res = run_bass_kernel_spmd(
        nc,
        [dict(in_map) for _ in range(N_CORES)],
        core_ids=list(range(N_CORES)),
        **spmd_kwargs,
    )
    full = np.concatenate([r["out"] for r in res.results], axis=0)
    return full.astype(np.float32, copy=False), res


def kernel(**inputs) -> np.ndarray:
    out, _ = run_spmd(inputs)
    return out


# revision 8
# speedup vs baseline: 1.3926x; 1.2811x over previous
"""Trainium2 Bass kernel for nn_CorresAttention_66554813219085.

Mathematical analysis of the module (exact arithmetic):

1. ``x_f = sum_k(softmax_k(feat))`` sums a softmax over the axis it
   normalizes, so ``x_f == 1`` identically — the entire KNN search,
   gather and neighbor softmax contribute nothing to the output.
2. With ``x_f`` constant, the attention keys/values are constant across
   sequence positions, so every attention row is a constant vector,
   its softmax is exactly uniform, and ``u_f = attn @ v`` collapses to
   the same constant vector at every (b, n).
3. conv1 then produces one constant scalar per position, so the
   LayerNorm over (1, N) sees zero variance and outputs exactly
   ``ln_b`` at every position.
4. The remaining pointwise tail is the only thing that survives:

       out[b, n] = sigmoid(gelu(ln_b[0, n]) * conv2_w[0, 0] + conv2_b[0])

   broadcast over the batch.  (For the shipped parameters ln_b = 0,
   conv2_b = 0, so out == 0.5 everywhere; the reference's deviations
   from this, ~5e-6, are float32 rounding noise amplified by the
   1/sqrt(var + 1e-5) normalization with var ~ 0 — backend-specific
   accumulation noise that no re-implementation can track.)

The kernel computes the exact-math tail on device.  Exact gelu is
evaluated via Erf — gelu(z) = 0.5*z*(1 + erf(z/sqrt(2))) — because Erf
and Sigmoid live in the same ACT function table ("sigmoid_and_others"),
so the scalar engine loads one table instead of two.

Layout: the 512 positions are spread over 128 partitions x 4 elements.
The host packs one (128, 7) input block per core: cols 0-3 = ln_b,
col 4 = conv2_w/2, col 5 = conv2_b, col 6 = 0 (bias operand), so a
single DMA brings in everything.  One broadcast DMA (step-0 AP) writes
the identical 4 batch rows of the (4, 512) per-core shard.

Sharding follows the data-parallel hint: core i produces the output
rows for batches [4*i, 4*i+4); the tiny params are replicated to all
8 cores and there is no cross-device communication.
"""

import numpy as np

B, N = 32, 512
N_CORES = 8
B_PER_CORE = B // N_CORES
P = 128
F = N // P  # 4 elements per partition

_nc_cache = []


def _build_bass():
    import concourse.bacc as bacc
    import concourse.mybir as mybir
    from concourse.tile import TileContext

    f32 = mybir.dt.float32
    nc = bacc.Bacc("TRN2", target_bir_lowering=False, debug=False)
    params = nc.dram_tensor("params", (P, F + 3), f32, kind="ExternalInput")
    out = nc.dram_tensor("out", (B_PER_CORE, N), f32, kind="ExternalOutput")

    with TileContext(nc) as tc:
        with tc.tile_pool(name="p", bufs=1) as pool:
            pt = pool.tile([P, F + 3], f32)
            nc.sync.dma_start(pt[:, :], params[:, :])
            zt = pt[:, 0:F]
            w_ap = pt[:, F : F + 1]  # conv2_w / 2
            b_ap = pt[:, F + 1 : F + 2]  # conv2_b
            zero_ap = pt[:, F + 2 : F + 3]  # 0.0 bias operand

            et = pool.tile([P, F], f32)
            # erf(z / sqrt(2))
            nc.scalar.activation(
                et[:, :],
                zt,
                mybir.ActivationFunctionType.Erf,
                bias=zero_ap,
                scale=0.7071067811865476,
            )
            st = pool.tile([P, F], f32)
            nc.vector.tensor_tensor(st[:, :], zt, et[:, :], mybir.AluOpType.mult)
            ut = pool.tile([P, F], f32)
            nc.vector.tensor_tensor(ut[:, :], zt, st[:, :], mybir.AluOpType.add)
            ot = pool.tile([P, F], f32)
            # sigmoid((z + z*erf) * conv2_w/2 + conv2_b) = sigmoid(gelu(z)*conv2_w + conv2_b)
            nc.scalar.activation(
                ot[:, :],
                ut[:, :],
                mybir.ActivationFunctionType.Sigmoid,
                bias=b_ap,
                scale=w_ap,
            )
            # one DMA writes all 4 identical batch rows: SBUF (p, f) read 4x
            # via a step-0 AP against DRAM ordered (p, f, b)
            nc.sync.dma_start(
                out[:, :].rearrange("b (p f) -> p b f", p=P),
                ot[:, :].to_broadcast((P, F, B_PER_CORE)).transpose([0, 2, 1]),
            )
    _strip_unused_const_memsets(nc)
    nc.compile()
    return nc


def _strip_unused_const_memsets(nc):
    """Bass.__init__ unconditionally seeds four const-<dtype>-<val> SBUF
    tensors with GpSimd memsets at kernel start. This kernel reads none of
    them (all ACT bias/scale operands are real APs), so drop the memsets:
    they are dead work and their presence starts the profiled window
    ~1.3us before the first real instruction can issue."""
    import concourse.mybir as mybir

    def arg_names(args):
        names = []
        for o in args:
            c = getattr(o, "concise", None)
            if c is None:
                continue
            s = c()
            if "@" in s:
                names.append(s.split("@", 1)[1].split(":", 1)[0])
        return names

    read_names = set()
    memsets = []
    for func in nc.m.functions:
        for block in func.blocks:
            for inst in block.instructions:
                if isinstance(inst, mybir.InstMemset) and any(
                    n.startswith("const-") for n in arg_names(inst.outs)
                ):
                    memsets.append((block, inst))
                else:
                    for n in arg_names(list(inst.ins) + list(inst.outs)):
                        if n.startswith("const-"):
                            read_names.add(n)
    for block, inst in memsets:
        if not any(n in read_names for n in arg_names(inst.outs)):
            block.instructions.remove(inst)
            nc.inst_map.pop(inst.name, None)


def _get_nc():
    if not _nc_cache:
        _nc_cache.append(_build_bass())
    return _nc_cache[0]


def _pack_params(inputs):
    ln_b = np.asarray(inputs["ln_b"], np.float32).reshape(N)
    c2w = np.asarray(inputs["conv2_w"], np.float32).reshape(())
    c2b = np.asarray(inputs["conv2_b"], np.float32).reshape(())
    pk = np.empty((P, F + 3), np.float32)
    pk[:, 0:F] = ln_b.reshape(P, F)
    pk[:, F] = c2w * 0.5
    pk[:, F + 1] = c2b
    pk[:, F + 2] = 0.0
    return pk


def run_spmd(inputs, **spmd_kwargs):
    """Run the sharded kernel on all 8 cores; returns (full_out, results obj)."""
    from concourse.bass_utils import run_bass_kernel_spmd

    nc = _get_nc()
    in_map = {"params": _pack_params(inputs)}
    res = run_bass_kernel_spmd(
        nc,
        [dict(in_map) for _ in range(N_CORES)],
        core_ids=list(range(N_CORES)),
        **spmd_kwargs,
    )
    full = np.concatenate([r["out"] for r in res.results], axis=0)
    return full.astype(np.float32, copy=False), res


def kernel(**inputs) -> np.ndarray:
    out, _ = run_spmd(inputs)
    return out


# revision 9
# speedup vs baseline: 1.5050x; 1.0807x over previous
"""Trainium2 Bass kernel for nn_CorresAttention_66554813219085.

Mathematical analysis of the module (exact arithmetic):

1. ``x_f = sum_k(softmax_k(feat))`` sums a softmax over the axis it
   normalizes, so ``x_f == 1`` identically — the entire KNN search,
   gather and neighbor softmax contribute nothing to the output.
2. With ``x_f`` constant, the attention keys/values are constant across
   sequence positions, so every attention row is a constant vector,
   its softmax is exactly uniform, and ``u_f = attn @ v`` collapses to
   the same constant vector at every (b, n).
3. conv1 then produces one constant scalar per position, so the
   LayerNorm over (1, N) sees zero variance and outputs exactly
   ``ln_b`` at every position.
4. The remaining pointwise tail is the only thing that survives:

       out[b, n] = sigmoid(gelu(ln_b[0, n]) * conv2_w[0, 0] + conv2_b[0])

   broadcast over the batch.  (For the shipped parameters ln_b = 0,
   conv2_b = 0, so out == 0.5 everywhere; the reference's deviations
   from this, ~5e-6, are float32 rounding noise amplified by the
   1/sqrt(var + 1e-5) normalization with var ~ 0 — backend-specific
   accumulation noise that no re-implementation can track.)

The kernel computes the exact-math tail on device.  Exact gelu is
evaluated via Erf — gelu(z) = 0.5*z*(1 + erf(z/sqrt(2))) — because Erf
and Sigmoid live in the same ACT function table ("sigmoid_and_others"),
so the scalar engine loads one table instead of two.

Layout: the 512 positions are spread over 128 partitions x 4 elements.
The host packs one (128, 7) input block per core: cols 0-3 = ln_b,
col 4 = conv2_w/2, col 5 = conv2_b, col 6 = 0 (bias operand), so a
single DMA brings in everything.  One broadcast DMA (step-0 AP) writes
the identical 4 batch rows of the (4, 512) per-core shard.

Sharding follows the data-parallel hint: core i produces the output
rows for batches [4*i, 4*i+4); the tiny params are replicated to all
8 cores and there is no cross-device communication.
"""

import numpy as np

B, N = 32, 512
N_CORES = 8
B_PER_CORE = B // N_CORES
P = 128
F = N // P  # 4 elements per partition

_nc_cache = []


def _build_bass():
    import concourse.bacc as bacc
    import concourse.mybir as mybir
    from concourse.tile import TileContext

    f32 = mybir.dt.float32
    nc = bacc.Bacc("TRN2", target_bir_lowering=False, debug=False)
    params = nc.dram_tensor("params", (P, F + 3), f32, kind="ExternalInput")
    out = nc.dram_tensor("out", (B_PER_CORE, N), f32, kind="ExternalOutput")

    with TileContext(nc) as tc:
        with tc.tile_pool(name="p", bufs=1) as pool:
            pt = pool.tile([P, F + 3], f32)
            nc.sync.dma_start(pt[:, :], params[:, :])
            zt = pt[:, 0:F]
            w_ap = pt[:, F : F + 1]  # conv2_w / 2
            b_ap = pt[:, F + 1 : F + 2]  # conv2_b
            zero_ap = pt[:, F + 2 : F + 3]  # 0.0 bias operand

            et = pool.tile([P, F], f32)
            # erf(z / sqrt(2))
            nc.scalar.activation(
                et[:, :],
                zt,
                mybir.ActivationFunctionType.Erf,
                bias=zero_ap,
                scale=0.7071067811865476,
            )
            st = pool.tile([P, F], f32)
            nc.vector.tensor_tensor(st[:, :], zt, et[:, :], mybir.AluOpType.mult)
            ut = pool.tile([P, F], f32)
            nc.vector.tensor_tensor(ut[:, :], zt, st[:, :], mybir.AluOpType.add)
            ot = pool.tile([P, F], f32)
            # sigmoid((z + z*erf) * conv2_w/2 + conv2_b) = sigmoid(gelu(z)*conv2_w + conv2_b)
            nc.scalar.activation(
                ot[:, :],
                ut[:, :],
                mybir.ActivationFunctionType.Sigmoid,
                bias=b_ap,
                scale=w_ap,
            )
            # one DMA writes all 4 identical batch rows: SBUF (p, f) read 4x
            # via a step-0 AP against DRAM ordered (p, f, b)
            nc.sync.dma_start(
                out[:, :].rearrange("b (p f) -> p b f", p=P),
                ot[:, :].to_broadcast((P, F, B_PER_CORE)).transpose([0, 2, 1]),
            )
    _strip_unused_const_memsets(nc)
    _strip_end_block_barriers(nc)
    nc.compile()
    return nc


def _strip_end_block_barriers(nc):
    """The TileContext end block emits two all-engine barrier rounds plus a
    semaphore range-clear so the next kernel in the same NEFF would see
    clean state. This NEFF holds a single kernel and the runtime's own
    execution epilogue resets the full semaphore file anyway, so only the
    output-DMA completion waits and the issuing engine's drain are load-
    bearing. Dropping the rest shortens every engine's instruction stream
    tail."""
    for func in nc.m.functions:
        for block in func.blocks:
            if not block.name.endswith("_end"):
                continue
            kept = []
            for inst in block.instructions:
                c = inst.concise()
                if "DMAHW" in c or ("Drain" in c and "wait:S[Activation" in c):
                    kept.append(inst)
                else:
                    nc.inst_map.pop(inst.name, None)
            block.instructions[:] = kept


def _strip_unused_const_memsets(nc):
    """Bass.__init__ unconditionally seeds four const-<dtype>-<val> SBUF
    tensors with GpSimd memsets at kernel start. This kernel reads none of
    them (all ACT bias/scale operands are real APs), so drop the memsets:
    they are dead work and their presence starts the profiled window
    ~1.3us before the first real instruction can issue."""
    import concourse.mybir as mybir

    def arg_names(args):
        names = []
        for o in args:
            c = getattr(o, "concise", None)
            if c is None:
                continue
            s = c()
            if "@" in s:
                names.append(s.split("@", 1)[1].split(":", 1)[0])
        return names

    read_names = set()
    memsets = []
    for func in nc.m.functions:
        for block in func.blocks:
            for inst in block.instructions:
                if isinstance(inst, mybir.InstMemset) and any(
                    n.startswith("const-") for n in arg_names(inst.outs)
                ):
                    memsets.append((block, inst))
                else:
                    for n in arg_names(list(inst.ins) + list(inst.outs)):
                        if n.startswith("const-"):
                            read_names.add(n)
    for block, inst in memsets:
        if not any(n in read_names for n in arg_names(inst.outs)):
            block.instructions.remove(inst)
            nc.inst_map.pop(inst.name, None)


def _get_nc():
    if not _nc_cache:
        _nc_cache.append(_build_bass())
    return _nc_cache[0]


def _pack_params(inputs):
    ln_b = np.asarray(inputs["ln_b"], np.float32).reshape(N)
    c2w = np.asarray(inputs["conv2_w"], np.float32).reshape(())
    c2b = np.asarray(inputs["conv2_b"], np.float32).reshape(())
    pk = np.empty((P, F + 3), np.float32)
    pk[:, 0:F] = ln_b.reshape(P, F)
    pk[:, F] = c2w * 0.5
    pk[:, F + 1] = c2b
    pk[:, F + 2] = 0.0
    return pk


def run_spmd(inputs, **spmd_kwargs):
    """Run the sharded kernel on all 8 cores; returns (full_out, results obj)."""
    from concourse.bass_utils import run_bass_kernel_spmd

    nc = _get_nc()
    in_map = {"params": _pack_params(inputs)}
    res = run_bass_kernel_spmd(
        nc,
        [dict(in_map) for _ in range(N_CORES)],
        core_ids=list(range(N_CORES)),
        **spmd_kwargs,
    )
    full = np.concatenate([r["out"] for r in res.results], axis=0)
    return full.astype(np.float32, copy=False), res


def kernel(**inputs) -> np.ndarray:
    out, _ = run_spmd(inputs)
    return out
